# revision 25
# baseline (speedup 1.0000x reference)
"""Trainium2 Bass kernel for the YAT MixerBlock (nn_MixerBlock_12524124635797).

Data-parallel over batch (64 -> 8 per core); all four GEMMs run as fp8e4
DoubleRow matmuls (2 K-chunks per instruction = 2x fp16 PE throughput).

Scaling scheme (power-of-2 scales keep fp8/fp16 ranges healthy; exact
compensation happens in fp32 psum / affine ops):
  tw8 = q8(-64*tw), cw8 = q8(-64*cw)       -> psum_dot = -64*dot
  den ops produce 32*den = psum + 32*(wn+eps) + 32*xn   [DVE affine]
  rec = 1/(32*den)                                       [DVE recip]
  sq  = Square(-0.5*psum + 32*bias) = 1024*(dot+b)^2     [ACT]
  h8  = sq*rec = 32*h  (fp8)                             [Pool mul]
  w2s8 = q8(2*scale_t*w2), w4s8 = q8(2*scale_c*w4)
  x2T = psum/(32*2) + (x.T + b2)                         [DVE affine]
  out = psum/(32*2) + x2T  (+64*b4 folded into the GEMM via fp8 row)

Per-core layout: token stage works per batch in (p-part, c) orientation;
channel stage works entirely transposed (c-part, rows=b*196+p free), the
final output is written as outT (768 x 1568) and transposed on host.
"""

import numpy as np
import ml_dtypes

import concourse.bass as bass
import concourse.bacc as bacc
import concourse.mybir as mybir
from concourse import bass_utils
from concourse import tile

F8 = mybir.dt.float8e4
F16 = mybir.dt.float16
F32 = mybir.dt.float32
AF = mybir.ActivationFunctionType
DR = mybir.MatmulPerfMode.DoubleRow

EPS = 0.1
B, P, C, T, M3 = 64, 196, 768, 384, 3072
NCORES = 8
BL = B // NCORES          # 8 batches per core
ROWS = BL * P             # 1568 rows per core
RB = 392                  # channel row-block (4 equal blocks)
NBLK = ROWS // RB


def build_program():
    nc = bacc.Bacc(
        "TRN2",
        target_bir_lowering=False,
        debug=False,
        enable_asserts=False,
        num_devices=NCORES,
    )

    d = {}
    d["x8"] = nc.dram_tensor("x8", [BL, 128, 2, C], F8, kind="ExternalInput").ap()
    d["tw8c"] = nc.dram_tensor("tw8c", [128, 2, T], F8, kind="ExternalInput").ap()
    d["tw8a"] = nc.dram_tensor("tw8a", [128, 2, T], F8, kind="ExternalInput").ap()
    d["tb32"] = nc.dram_tensor("tb32", [128, 3], F32, kind="ExternalInput").ap()
    d["w2s8"] = nc.dram_tensor("w2s8", [128, 4, P], F8, kind="ExternalInput").ap()
    d["xTp"] = nc.dram_tensor("xTp", [128, 6, ROWS], F16, kind="ExternalInput").ap()
    d["cw8"] = nc.dram_tensor("cw8", [128, 6, M3], F8, kind="ExternalInput").ap()
    d["wncS"] = nc.dram_tensor("wncS", [128, 24], F32, kind="ExternalInput").ap()
    d["cb32"] = nc.dram_tensor("cb32", [128, 24], F32, kind="ExternalInput").ap()
    d["w4s8"] = nc.dram_tensor("w4s8", [128, 24, C], F8, kind="ExternalInput").ap()
    d["b4s8"] = nc.dram_tensor("b4s8", [1, C], F8, kind="ExternalInput").ap()
    out_dram = nc.dram_tensor("outT", [C, ROWS], F16, kind="ExternalOutput").ap()

    with tile.TileContext(nc) as tc:
        with tc.tile_pool(name="consts", bufs=1) as cp:
            tw8c = cp.tile([128, 2, T], F8)
            tw8a = cp.tile([128, 2, T], F8)
            tb32 = cp.tile([128, 3], F32)
            w2s8 = cp.tile([128, 4, P], F8)
            xTp = cp.tile([128, 6, ROWS], F16)
            cw8 = cp.tile([128, 6, M3], F8)
            wncS = cp.tile([128, 24], F32)
            cb32 = cp.tile([128, 24], F32)
            w4s8 = cp.tile([128, 24, C], F8)
            b4s8 = cp.tile([1, C], F8)
            ones32 = cp.tile([128, 128], F16)
            one8 = cp.tile([1, 512], F8)
            x2T = cp.tile([128, 6, ROWS], F16)
            x2T8 = cp.tile([128, 6, ROWS], F8)

            # token-critical loads first on the sync queue
            nc.sync.dma_start(tw8c[:], d["tw8c"])
            nc.sync.dma_start(tw8a[:], d["tw8a"])
            xbs = [cp.tile([128, 2, C], F8, name=f"xb{b}") for b in range(BL)]
            nc.sync.dma_start(xbs[0][:], d["x8"][0])
            nc.sync.dma_start(tb32[:], d["tb32"])
            nc.sync.dma_start(xbs[1][:], d["x8"][1])
            nc.sync.dma_start(w2s8[:], d["w2s8"])
            for b in range(2, BL):
                nc.sync.dma_start(xbs[b][:], d["x8"][b])
            nc.sync.dma_start(xTp[:], d["xTp"])
            # channel weights on the scalar-engine HWDGE queue
            nc.scalar.dma_start(cw8[:], d["cw8"])
            nc.scalar.dma_start(w4s8[:], d["w4s8"])
            nc.scalar.dma_start(b4s8[:], d["b4s8"])
            nc.sync.dma_start(wncS[:], d["wncS"])
            nc.sync.dma_start(cb32[:], d["cb32"])
            nc.vector.memset(ones32[:], 32.0)
            nc.vector.memset(one8[:], 1.0)

            # ================= Token stage =================
            # Software-pipelined over batches: batch b's dot1 GEMMs issue on
            # the PE before batch b-1's linear GEMMs, so the in-order PE
            # queue never waits on the DVE/ACT/Pool yat chain.
            with (
                tc.tile_pool(name="tok_sbuf", bufs=2) as tp,
                tc.tile_pool(name="tok_psum", bufs=1, space="PSUM") as pp,
            ):
                def tok_front(b):
                    """dual dot1 (clean + norm-augmented) + yat chain -> h8.

                    tw8a carries 3 extra fp8 K-rows in dot1's zero-pad space
                    (xn coarse, xn residual, 32*(wnt+eps)), so the augmented
                    psum is 32*den directly and no DVE den op is needed."""
                    xb = xbs[b]
                    h8 = tp.tile([128, 4, C], F8, tag="h8")
                    # chunk 3 pairs with w2s8's zero chunk; garbage fp8 NaN
                    # bits would still poison 0*NaN -> zero it.
                    nc.gpsimd.memset(h8[:, 3, :], 0.0)
                    rec3 = tp.tile([128, 3, C], F32, tag="rec3")
                    sq3 = tp.tile([128, 3, C], F16, tag="sq3")
                    for tcn in range(3):
                        ps1a = pp.tile([128, C], F32, tag="ps1a", bufs=2)
                        for no, nn_ in ((0, 512), (512, 256)):
                            nc.tensor.matmul(
                                ps1a[:, no : no + nn_],
                                tw8a[:, 0:2, tcn * 128 : (tcn + 1) * 128],
                                xb[:, 0:2, no : no + nn_],
                                start=True, stop=True, perf_mode=DR,
                            )
                        nc.vector.reciprocal_approx_fast(
                            rec3[:, tcn, :], ps1a[:]
                        )
                        ps1c = pp.tile([128, C], F32, tag="ps1c", bufs=1)
                        for no, nn_ in ((0, 512), (512, 256)):
                            nc.tensor.matmul(
                                ps1c[:, no : no + nn_],
                                tw8c[:, 0:2, tcn * 128 : (tcn + 1) * 128],
                                xb[:, 0:2, no : no + nn_],
                                start=True, stop=True, perf_mode=DR,
                            )
                        nc.scalar.activation(
                            sq3[:, tcn, :], ps1c[:], AF.Square,
                            bias=tb32[:, tcn : tcn + 1], scale=-0.5,
                        )
                    nc.gpsimd.tensor_mul(
                        h8[:, 0:2, :].rearrange("p a b -> p (a b)"),
                        sq3[:, 0:2, :].rearrange("p a b -> p (a b)"),
                        rec3[:, 0:2, :].rearrange("p a b -> p (a b)"),
                    )
                    nc.gpsimd.tensor_mul(h8[:, 2, :], sq3[:, 2, :], rec3[:, 2, :])
                    return h8

                def tok_back(b, h8):
                    """token linear GEMMs + shortcut affine for batch b."""
                    r0 = b * P
                    for mc in range(6):
                        ps2 = pp.tile([128, P], F32, tag="ps2", bufs=2)
                        for j in range(2):
                            nc.tensor.matmul(
                                ps2[:],
                                h8[:, 2 * j : 2 * j + 2, mc * 128 : (mc + 1) * 128],
                                w2s8[:, 2 * j : 2 * j + 2, :],
                                start=(j == 0), stop=(j == 1), perf_mode=DR,
                            )
                        nc.vector.affine_then_add(
                            x2T[:, mc, r0 : r0 + P], ps2[:],
                            xTp[:, mc, r0 : r0 + P],
                            scale=1.0 / 64.0, bias=0.0,
                        )
                        nc.scalar.copy(
                            x2T8[:, mc, r0 : r0 + P], x2T[:, mc, r0 : r0 + P]
                        )

                prev = tok_front(0)
                for b in range(1, BL):
                    cur = tok_front(b)
                    tok_back(b - 1, prev)
                    prev = cur
                tok_back(BL - 1, prev)

            # ================= Channel stage =================
            with (
                tc.tile_pool(name="ch_sbuf", bufs=2) as chp,
                tc.tile_pool(name="ch_psum", bufs=1, space="PSUM") as cpp,
            ):
                for blk in range(NBLK):
                    r0 = blk * RB
                    # 32*row-norms: Pool squares + ones(32) GEMM
                    ps_xn2 = cpp.tile([128, RB], F32, tag="ps_d2", bufs=2)
                    x2sq = chp.tile([128, 6, RB], F16, tag="x2sq", bufs=2)
                    for kc in range(6):
                        nc.scalar.activation(
                            x2sq[:, kc, :], x2T[:, kc, r0 : r0 + RB],
                            AF.Square, scale=1.0,
                        )
                        nc.tensor.matmul(
                            ps_xn2[:],
                            ones32[:, :],
                            x2sq[:, kc, :],
                            start=(kc == 0), stop=(kc == 5),
                        )
                    xnb2 = chp.tile([128, RB], F32, tag="xnb2", bufs=2)
                    nc.scalar.copy(xnb2[:], ps_xn2[:])

                    po = [
                        cpp.tile([128, RB], F32, tag=f"po{s_}", bufs=1,
                                 name=f"po{s_}")
                        for s_ in range(6)
                    ]
                    for j in range(12):
                        h8p = chp.tile([128, 2, RB], F8, tag="h8p", bufs=4)
                        for i in range(2):
                            mc = 2 * j + i
                            ps_d2 = cpp.tile([128, RB], F32, tag="ps_d2",
                                             bufs=2)
                            for k in range(3):
                                nc.tensor.matmul(
                                    ps_d2[:],
                                    cw8[:, 2 * k : 2 * k + 2,
                                        mc * 128 : (mc + 1) * 128],
                                    x2T8[:, 2 * k : 2 * k + 2, r0 : r0 + RB],
                                    start=(k == 0), stop=(k == 2),
                                    perf_mode=DR,
                                )
                            den2 = chp.tile([128, RB], F32, tag="den2", bufs=6)
                            nc.vector.affine_then_add(
                                den2[:], ps_d2[:], xnb2[:],
                                scale=1.0, bias=wncS[:, mc : mc + 1],
                            )
                            rec2 = chp.tile([128, RB], F32, tag="rec2", bufs=6)
                            nc.vector.reciprocal_approx_fast(rec2[:], den2[:])
                            sq2 = chp.tile([128, RB], F16, tag="sq2", bufs=6)
                            nc.scalar.activation(
                                sq2[:], ps_d2[:], AF.Square,
                                bias=cb32[:, mc : mc + 1], scale=-0.5,
                            )
                            nc.gpsimd.tensor_mul(h8p[:, i, :], sq2[:], rec2[:])
                        for cc in range(6):
                            nc.tensor.matmul(
                                po[cc][:],
                                w4s8[:, 2 * j : 2 * j + 2,
                                     cc * 128 : (cc + 1) * 128],
                                h8p[:, 0:2, :],
                                start=(j == 0), stop=False, perf_mode=DR,
                            )
                    for cc in range(6):
                        # +64*b4 via fp8 K=1 row, closes the accumulation
                        nc.tensor.matmul(
                            po[cc][:],
                            b4s8[0:1, cc * 128 : (cc + 1) * 128],
                            one8[0:1, 0:RB],
                            start=False, stop=True,
                        )
                        o16 = chp.tile([128, RB], F16, tag="o16", bufs=3)
                        nc.vector.affine_then_add(
                            o16[:], po[cc][:], x2T[:, cc, r0 : r0 + RB],
                            scale=1.0 / 64.0, bias=0.0,
                        )
                        nc.sync.dma_start(
                            out_dram[cc * 128 : (cc + 1) * 128, r0 : r0 + RB],
                            o16[:],
                        )

    nc.compile()
    return nc


_Q8 = ml_dtypes.float8_e4m3


def _q8(a):
    return np.asarray(a, np.float32).astype(_Q8)


_PROGRAM = None


def _get_program():
    global _PROGRAM
    if _PROGRAM is None:
        _PROGRAM = build_program()
    return _PROGRAM


def kernel(x, tw, tb, t_alpha, w2, b2, cw, cb, c_alpha, w4, b4, _trace=False):
    x = np.asarray(x, np.float32)
    tw = np.asarray(tw, np.float32)
    tb = np.asarray(tb, np.float32)
    w2 = np.asarray(w2, np.float32)
    b2 = np.asarray(b2, np.float32)
    cw = np.asarray(cw, np.float32)
    cb = np.asarray(cb, np.float32)
    w4 = np.asarray(w4, np.float32)
    b4 = np.asarray(b4, np.float32)

    scale_t = np.float32(np.sqrt(np.float32(T / np.log(T + 1.0)))) ** np.asarray(
        t_alpha, np.float32
    )[0]
    scale_c = np.float32(np.sqrt(np.float32(M3 / np.log(M3 + 1.0)))) ** np.asarray(
        c_alpha, np.float32
    )[0]

    # ---- shared weight packs ----
    # tw8c[p, kc, t] = q8(-64*tw[t, kc*128+p]); tw8a adds 3 norm rows in
    # the zero-pad space of K-chunk 1 (rows 68..70).
    tw8c = np.zeros((128, 2, T), np.float32)
    tw8c[0:128, 0, :] = -64.0 * tw[:, 0:128].T
    tw8c[0:68, 1, :] = -64.0 * tw[:, 128:196].T
    wn_t = (tw ** 2).sum(1) + EPS
    tw8a = tw8c.copy()
    tw8a[68, 1, :] = 64.0
    tw8a[69, 1, :] = 4.0
    tw8a[70, 1, :] = 1.0
    tw8a[71, 1, :] = np.asarray(_q8(32.0 * wn_t), np.float32)
    tb32 = 32.0 * tb.reshape(3, 128).T
    w2s = 2.0 * scale_t * w2  # (P, T)
    w2s8 = np.zeros((128, 4, P), np.float32)
    for kc in range(3):
        w2s8[:, kc, :] = w2s.T[kc * 128 : (kc + 1) * 128, :]
    cw8 = np.zeros((128, 6, M3), np.float32)
    for kc in range(6):
        cw8[:, kc, :] = -64.0 * cw[:, kc * 128 : (kc + 1) * 128].T
    wn_c = (cw ** 2).sum(1) + EPS
    wncS = 32.0 * wn_c.reshape(24, 128).T
    cb32 = 32.0 * cb.reshape(24, 128).T
    w4s = 2.0 * scale_c * w4  # (C, M3)
    w4s8 = np.zeros((128, 24, C), np.float32)
    for mc in range(24):
        w4s8[:, mc, :] = w4s.T[mc * 128 : (mc + 1) * 128, :]
    b4s8 = (64.0 * b4).reshape(1, C)

    shared = {
        "tw8c": _q8(tw8c),
        "tw8a": _q8(tw8a),
        "tb32": np.ascontiguousarray(tb32.astype(np.float32)),
        "w2s8": _q8(w2s8),
        "cw8": _q8(cw8),
        "wncS": np.ascontiguousarray(wncS.astype(np.float32)),
        "cb32": np.ascontiguousarray(cb32.astype(np.float32)),
        "w4s8": _q8(w4s8),
        "b4s8": _q8(b4s8),
    }

    # ---- per-core activations ----
    xr = x.reshape(NCORES, BL, P, C)
    x8 = np.zeros((NCORES, BL, 128, 2, C), np.float32)
    x8[:, :, 0:128, 0, :] = xr[:, :, 0:128, :]
    x8[:, :, 0:68, 1, :] = xr[:, :, 128:196, :]
    # norm rows for the augmented dot1: a 3-level fp8 residual ladder gets
    # 32*xn into the psum at ~0.03% error; row 71 adds 32*(wnt+eps).
    xn1 = (xr.astype(np.float32) ** 2).sum(axis=2)      # (NC, BL, C)
    xn_c = np.asarray(_q8(xn1 / 2.0), np.float32)
    r1 = 32.0 * xn1 - 64.0 * xn_c
    xn_m = np.asarray(_q8(r1 / 4.0), np.float32)
    r2 = r1 - 4.0 * xn_m
    x8[:, :, 68, 1, :] = xn_c
    x8[:, :, 69, 1, :] = xn_m
    x8[:, :, 70, 1, :] = r2
    x8[:, :, 71, 1, :] = 1.0
    x8 = _q8(x8)
    # xTp[cq, cc, b*196+p] = x[b, p, cc*128+cq] + b2[p]
    xt = xr.transpose(0, 3, 1, 2).reshape(NCORES, C, ROWS) + np.tile(
        b2, BL
    )[None, None, :]
    xTp = xt.reshape(NCORES, 6, 128, ROWS).transpose(0, 2, 1, 3).astype(np.float16)

    in_maps = [
        dict(shared, x8=x8[c], xTp=np.ascontiguousarray(xTp[c]))
        for c in range(NCORES)
    ]

    nc = _get_program()
    kwargs = {}
    if _trace:
        import os
        import shutil

        shutil.rmtree("/tmp/bass_ntff", ignore_errors=True)
        os.makedirs("/tmp/bass_ntff", exist_ok=True)
        kwargs["tmpdir"] = "/tmp/bass_ntff"
    res = bass_utils.run_bass_kernel_spmd(
        nc, in_maps, core_ids=list(range(NCORES)), trace=_trace, **kwargs
    )
    # outT (C, ROWS) fp16 -> (BL, P, C) fp32 per core
    outs = []
    for c in range(NCORES):
        oT = np.asarray(res.results[c]["outT"], np.float32)   # (768, 1568)
        outs.append(oT.reshape(C, BL, P).transpose(1, 2, 0))
    out = np.concatenate(outs, axis=0).reshape(B, P, C)
    if _trace:
        kernel.last_results = res
    return out


# revision 26
# speedup vs baseline: 1.1393x; 1.1393x over previous
"""Trainium2 Bass kernel for the YAT MixerBlock (nn_MixerBlock_12524124635797).

Data-parallel over batch (64 -> 8 per core); all four GEMMs run as fp8e4
DoubleRow matmuls (2 K-chunks per instruction = 2x fp16 PE throughput).

Scaling scheme (power-of-2 scales keep fp8/fp16 ranges healthy; exact
compensation happens in fp32 psum / affine ops):
  tw8 = q8(-64*tw), cw8 = q8(-64*cw)       -> psum_dot = -64*dot
  den ops produce 32*den = psum + 32*(wn+eps) + 32*xn   [DVE affine]
  rec = 1/(32*den)                                       [DVE recip]
  sq  = Square(-0.5*psum + 32*bias) = 1024*(dot+b)^2     [ACT]
  h8  = sq*rec = 32*h  (fp8)                             [Pool mul]
  w2s8 = q8(2*scale_t*w2), w4s8 = q8(2*scale_c*w4)
  x2T = psum/(32*2) + (x.T + b2)                         [DVE affine]
  out = psum/(32*2) + x2T  (+64*b4 folded into the GEMM via fp8 row)

Per-core layout: token stage works per batch in (p-part, c) orientation;
channel stage works entirely transposed (c-part, rows=b*196+p free), the
final output is written as outT (768 x 1568) and transposed on host.
"""

import numpy as np
import ml_dtypes

import concourse.bass as bass
import concourse.bacc as bacc
import concourse.mybir as mybir
from concourse import bass_utils
from concourse import tile

F8 = mybir.dt.float8e4
F16 = mybir.dt.float16
F32 = mybir.dt.float32
AF = mybir.ActivationFunctionType
DR = mybir.MatmulPerfMode.DoubleRow

EPS = 0.1
B, P, C, T, M3 = 64, 196, 768, 384, 3072
NCORES = 8
BL = B // NCORES          # 8 batches per core
ROWS = BL * P             # 1568 rows per core
RB = 392                  # channel row-block (4 equal blocks)
NBLK = ROWS // RB


def build_program():
    nc = bacc.Bacc(
        "TRN2",
        target_bir_lowering=False,
        debug=False,
        enable_asserts=False,
        num_devices=NCORES,
    )

    d = {}
    d["x8"] = nc.dram_tensor("x8", [BL, 128, 2, C], F8, kind="ExternalInput").ap()
    d["tw8c"] = nc.dram_tensor("tw8c", [128, 2, T], F8, kind="ExternalInput").ap()
    d["tw8a"] = nc.dram_tensor("tw8a", [128, 2, T], F8, kind="ExternalInput").ap()
    d["tb32"] = nc.dram_tensor("tb32", [128, 3], F32, kind="ExternalInput").ap()
    d["w2s8"] = nc.dram_tensor("w2s8", [128, 4, P], F8, kind="ExternalInput").ap()
    d["xTp"] = nc.dram_tensor("xTp", [128, 6, ROWS], F16, kind="ExternalInput").ap()
    d["cw8"] = nc.dram_tensor("cw8", [128, 6, M3], F8, kind="ExternalInput").ap()
    d["wncS"] = nc.dram_tensor("wncS", [128, 24], F32, kind="ExternalInput").ap()
    d["cb32"] = nc.dram_tensor("cb32", [128, 24], F32, kind="ExternalInput").ap()
    d["w4s8"] = nc.dram_tensor("w4s8", [128, 24, C], F8, kind="ExternalInput").ap()
    d["b4s8"] = nc.dram_tensor("b4s8", [1, C], F8, kind="ExternalInput").ap()
    out_dram = nc.dram_tensor("outT", [C, ROWS], F16, kind="ExternalOutput").ap()

    with tile.TileContext(nc) as tc:
        with tc.tile_pool(name="consts", bufs=1) as cp:
            tw8c = cp.tile([128, 2, T], F8)
            tw8a = cp.tile([128, 2, T], F8)
            tb32 = cp.tile([128, 3], F32)
            w2s8 = cp.tile([128, 4, P], F8)
            xTp = cp.tile([128, 6, ROWS], F16)
            cw8 = cp.tile([128, 6, M3], F8)
            wncS = cp.tile([128, 24], F32)
            cb32 = cp.tile([128, 24], F32)
            w4s8 = cp.tile([128, 24, C], F8)
            b4s8 = cp.tile([1, C], F8)
            ones32 = cp.tile([128, 128], F16)
            one8 = cp.tile([1, 512], F8)
            x2T = cp.tile([128, 6, ROWS], F16)
            x2T8 = cp.tile([128, 6, ROWS], F8)

            # token-critical loads first on the sync queue
            nc.sync.dma_start(tw8c[:], d["tw8c"])
            nc.sync.dma_start(tw8a[:], d["tw8a"])
            xbs = [cp.tile([128, 2, C], F8, name=f"xb{b}") for b in range(BL)]
            nc.sync.dma_start(xbs[0][:], d["x8"][0])
            nc.sync.dma_start(tb32[:], d["tb32"])
            nc.sync.dma_start(xbs[1][:], d["x8"][1])
            nc.sync.dma_start(w2s8[:], d["w2s8"])
            for b in range(2, BL):
                nc.sync.dma_start(xbs[b][:], d["x8"][b])
            nc.sync.dma_start(xTp[:], d["xTp"])
            # channel weights on the scalar-engine HWDGE queue
            nc.scalar.dma_start(cw8[:], d["cw8"])
            nc.scalar.dma_start(w4s8[:], d["w4s8"])
            nc.scalar.dma_start(b4s8[:], d["b4s8"])
            nc.sync.dma_start(wncS[:], d["wncS"])
            nc.sync.dma_start(cb32[:], d["cb32"])
            nc.vector.memset(ones32[:], 32.0)
            nc.vector.memset(one8[:], 1.0)

            # ================= Token stage =================
            # Software-pipelined over batches: batch b's dot1 GEMMs issue on
            # the PE before batch b-1's linear GEMMs, so the in-order PE
            # queue never waits on the DVE/ACT/Pool yat chain.
            with (
                tc.tile_pool(name="tok_sbuf", bufs=2) as tp,
                tc.tile_pool(name="tok_psum", bufs=1, space="PSUM") as pp,
            ):
                def tok_front(b):
                    """dual dot1 (clean + norm-augmented) + yat chain -> h8.

                    tw8a carries 3 extra fp8 K-rows in dot1's zero-pad space
                    (xn coarse, xn residual, 32*(wnt+eps)), so the augmented
                    psum is 32*den directly and no DVE den op is needed."""
                    xb = xbs[b]
                    h8 = tp.tile([128, 4, C], F8, tag="h8")
                    # chunk 3 pairs with w2s8's zero chunk; garbage fp8 NaN
                    # bits would still poison 0*NaN -> zero it.
                    nc.gpsimd.memset(h8[:, 3, :], 0.0)
                    rec3 = tp.tile([128, 3, C], F32, tag="rec3")
                    sq3 = tp.tile([128, 3, C], F16, tag="sq3")
                    for tcn in range(3):
                        ps1a = pp.tile([128, C], F32, tag="ps1a", bufs=2)
                        for no, nn_ in ((0, 512), (512, 256)):
                            nc.tensor.matmul(
                                ps1a[:, no : no + nn_],
                                tw8a[:, 0:2, tcn * 128 : (tcn + 1) * 128],
                                xb[:, 0:2, no : no + nn_],
                                start=True, stop=True, perf_mode=DR,
                            )
                        nc.vector.reciprocal_approx_fast(
                            rec3[:, tcn, :], ps1a[:]
                        )
                        ps1c = pp.tile([128, C], F32, tag="ps1c", bufs=1)
                        for no, nn_ in ((0, 512), (512, 256)):
                            nc.tensor.matmul(
                                ps1c[:, no : no + nn_],
                                tw8c[:, 0:2, tcn * 128 : (tcn + 1) * 128],
                                xb[:, 0:2, no : no + nn_],
                                start=True, stop=True, perf_mode=DR,
                            )
                        nc.scalar.activation(
                            sq3[:, tcn, :], ps1c[:], AF.Square,
                            bias=tb32[:, tcn : tcn + 1], scale=-0.5,
                        )
                    nc.gpsimd.tensor_mul(
                        h8[:, 0:2, :].rearrange("p a b -> p (a b)"),
                        sq3[:, 0:2, :].rearrange("p a b -> p (a b)"),
                        rec3[:, 0:2, :].rearrange("p a b -> p (a b)"),
                    )
                    nc.gpsimd.tensor_mul(h8[:, 2, :], sq3[:, 2, :], rec3[:, 2, :])
                    return h8

                def tok_back(b, h8):
                    """token linear GEMMs + shortcut affine for batch b."""
                    r0 = b * P
                    for mc in range(6):
                        ps2 = pp.tile([128, P], F32, tag="ps2", bufs=2)
                        for j in range(2):
                            nc.tensor.matmul(
                                ps2[:],
                                h8[:, 2 * j : 2 * j + 2, mc * 128 : (mc + 1) * 128],
                                w2s8[:, 2 * j : 2 * j + 2, :],
                                start=(j == 0), stop=(j == 1), perf_mode=DR,
                            )
                        nc.vector.affine_then_add(
                            x2T[:, mc, r0 : r0 + P], ps2[:],
                            xTp[:, mc, r0 : r0 + P],
                            scale=1.0 / 64.0, bias=0.0,
                        )
                        nc.scalar.copy(
                            x2T8[:, mc, r0 : r0 + P], x2T[:, mc, r0 : r0 + P]
                        )

                prev = tok_front(0)
                for b in range(1, BL):
                    cur = tok_front(b)
                    tok_back(b - 1, prev)
                    prev = cur
                tok_back(BL - 1, prev)

            # ================= Channel stage =================
            with (
                tc.tile_pool(name="ch_sbuf", bufs=2) as chp,
                tc.tile_pool(name="ch_psum", bufs=1, space="PSUM") as cpp,
            ):
                for blk in range(NBLK):
                    r0 = blk * RB
                    # 32*row-norms: Pool squares + ones(32) GEMM
                    ps_xn2 = cpp.tile([128, RB], F32, tag="ps_d2", bufs=2)
                    x2sq = chp.tile([128, 6, RB], F16, tag="x2sq", bufs=2)
                    for kc in range(6):
                        nc.gpsimd.tensor_mul(
                            x2sq[:, kc, :],
                            x2T[:, kc, r0 : r0 + RB],
                            x2T[:, kc, r0 : r0 + RB],
                        )
                        nc.tensor.matmul(
                            ps_xn2[:],
                            ones32[:, :],
                            x2sq[:, kc, :],
                            start=(kc == 0), stop=(kc == 5),
                        )
                    xnb2 = chp.tile([128, RB], F32, tag="xnb2", bufs=2)
                    nc.scalar.copy(xnb2[:], ps_xn2[:])

                    po = [
                        cpp.tile([128, RB], F32, tag=f"po{s_}", bufs=1,
                                 name=f"po{s_}")
                        for s_ in range(6)
                    ]
                    for j in range(12):
                        h8p = chp.tile([128, 2, RB], F8, tag="h8p", bufs=3)
                        for i in range(2):
                            mc = 2 * j + i
                            ps_d2 = cpp.tile([128, RB], F32, tag="ps_d2",
                                             bufs=2)
                            for k in range(3):
                                nc.tensor.matmul(
                                    ps_d2[:],
                                    cw8[:, 2 * k : 2 * k + 2,
                                        mc * 128 : (mc + 1) * 128],
                                    x2T8[:, 2 * k : 2 * k + 2, r0 : r0 + RB],
                                    start=(k == 0), stop=(k == 2),
                                    perf_mode=DR,
                                )
                            den2 = chp.tile([128, RB], F32, tag="den2", bufs=4)
                            nc.vector.affine_then_add(
                                den2[:], ps_d2[:], xnb2[:],
                                scale=1.0, bias=wncS[:, mc : mc + 1],
                            )
                            rec2 = chp.tile([128, RB], F32, tag="rec2", bufs=4)
                            nc.vector.reciprocal_approx_fast(rec2[:], den2[:])
                            sq2 = chp.tile([128, RB], F16, tag="sq2", bufs=4)
                            nc.scalar.activation(
                                sq2[:], ps_d2[:], AF.Square,
                                bias=cb32[:, mc : mc + 1], scale=-0.5,
                            )
                            nc.gpsimd.tensor_mul(h8p[:, i, :], sq2[:], rec2[:])
                        for cc in range(6):
                            nc.tensor.matmul(
                                po[cc][:],
                                w4s8[:, 2 * j : 2 * j + 2,
                                     cc * 128 : (cc + 1) * 128],
                                h8p[:, 0:2, :],
                                start=(j == 0), stop=False, perf_mode=DR,
                            )
                    for cc in range(6):
                        # +64*b4 via fp8 K=1 row, closes the accumulation
                        nc.tensor.matmul(
                            po[cc][:],
                            b4s8[0:1, cc * 128 : (cc + 1) * 128],
                            one8[0:1, 0:RB],
                            start=False, stop=True,
                        )
                        o16 = chp.tile([128, RB], F16, tag="o16", bufs=3)
                        nc.vector.affine_then_add(
                            o16[:], po[cc][:], x2T[:, cc, r0 : r0 + RB],
                            scale=1.0 / 64.0, bias=0.0,
                        )
                        nc.sync.dma_start(
                            out_dram[cc * 128 : (cc + 1) * 128, r0 : r0 + RB],
                            o16[:],
                        )

    nc.compile()
    return nc


_Q8 = ml_dtypes.float8_e4m3


def _q8(a):
    return np.asarray(a, np.float32).astype(_Q8)


_PROGRAM = None


def _get_program():
    global _PROGRAM
    if _PROGRAM is None:
        _PROGRAM = build_program()
    return _PROGRAM


def kernel(x, tw, tb, t_alpha, w2, b2, cw, cb, c_alpha, w4, b4, _trace=False):
    x = np.asarray(x, np.float32)
    tw = np.asarray(tw, np.float32)
    tb = np.asarray(tb, np.float32)
    w2 = np.asarray(w2, np.float32)
    b2 = np.asarray(b2, np.float32)
    cw = np.asarray(cw, np.float32)
    cb = np.asarray(cb, np.float32)
    w4 = np.asarray(w4, np.float32)
    b4 = np.asarray(b4, np.float32)

    scale_t = np.float32(np.sqrt(np.float32(T / np.log(T + 1.0)))) ** np.asarray(
        t_alpha, np.float32
    )[0]
    scale_c = np.float32(np.sqrt(np.float32(M3 / np.log(M3 + 1.0)))) ** np.asarray(
        c_alpha, np.float32
    )[0]

    # ---- shared weight packs ----
    # tw8c[p, kc, t] = q8(-64*tw[t, kc*128+p]); tw8a adds 3 norm rows in
    # the zero-pad space of K-chunk 1 (rows 68..70).
    tw8c = np.zeros((128, 2, T), np.float32)
    tw8c[0:128, 0, :] = -64.0 * tw[:, 0:128].T
    tw8c[0:68, 1, :] = -64.0 * tw[:, 128:196].T
    wn_t = (tw ** 2).sum(1) + EPS
    tw8a = tw8c.copy()
    tw8a[68, 1, :] = 64.0
    tw8a[69, 1, :] = 4.0
    tw8a[70, 1, :] = 1.0
    tw8a[71, 1, :] = np.asarray(_q8(32.0 * wn_t), np.float32)
    tb32 = 32.0 * tb.reshape(3, 128).T
    w2s = 2.0 * scale_t * w2  # (P, T)
    w2s8 = np.zeros((128, 4, P), np.float32)
    for kc in range(3):
        w2s8[:, kc, :] = w2s.T[kc * 128 : (kc + 1) * 128, :]
    cw8 = np.zeros((128, 6, M3), np.float32)
    for kc in range(6):
        cw8[:, kc, :] = -64.0 * cw[:, kc * 128 : (kc + 1) * 128].T
    wn_c = (cw ** 2).sum(1) + EPS
    wncS = 32.0 * wn_c.reshape(24, 128).T
    cb32 = 32.0 * cb.reshape(24, 128).T
    w4s = 2.0 * scale_c * w4  # (C, M3)
    w4s8 = np.zeros((128, 24, C), np.float32)
    for mc in range(24):
        w4s8[:, mc, :] = w4s.T[mc * 128 : (mc + 1) * 128, :]
    b4s8 = (64.0 * b4).reshape(1, C)

    shared = {
        "tw8c": _q8(tw8c),
        "tw8a": _q8(tw8a),
        "tb32": np.ascontiguousarray(tb32.astype(np.float32)),
        "w2s8": _q8(w2s8),
        "cw8": _q8(cw8),
        "wncS": np.ascontiguousarray(wncS.astype(np.float32)),
        "cb32": np.ascontiguousarray(cb32.astype(np.float32)),
        "w4s8": _q8(w4s8),
        "b4s8": _q8(b4s8),
    }

    # ---- per-core activations ----
    xr = x.reshape(NCORES, BL, P, C)
    x8 = np.zeros((NCORES, BL, 128, 2, C), np.float32)
    x8[:, :, 0:128, 0, :] = xr[:, :, 0:128, :]
    x8[:, :, 0:68, 1, :] = xr[:, :, 128:196, :]
    # norm rows for the augmented dot1: a 3-level fp8 residual ladder gets
    # 32*xn into the psum at ~0.03% error; row 71 adds 32*(wnt+eps).
    xn1 = (xr.astype(np.float32) ** 2).sum(axis=2)      # (NC, BL, C)
    xn_c = np.asarray(_q8(xn1 / 2.0), np.float32)
    r1 = 32.0 * xn1 - 64.0 * xn_c
    xn_m = np.asarray(_q8(r1 / 4.0), np.float32)
    r2 = r1 - 4.0 * xn_m
    x8[:, :, 68, 1, :] = xn_c
    x8[:, :, 69, 1, :] = xn_m
    x8[:, :, 70, 1, :] = r2
    x8[:, :, 71, 1, :] = 1.0
    x8 = _q8(x8)
    # xTp[cq, cc, b*196+p] = x[b, p, cc*128+cq] + b2[p]
    xt = xr.transpose(0, 3, 1, 2).reshape(NCORES, C, ROWS) + np.tile(
        b2, BL
    )[None, None, :]
    xTp = xt.reshape(NCORES, 6, 128, ROWS).transpose(0, 2, 1, 3).astype(np.float16)

    in_maps = [
        dict(shared, x8=x8[c], xTp=np.ascontiguousarray(xTp[c]))
        for c in range(NCORES)
    ]

    nc = _get_program()
    kwargs = {}
    if _trace:
        import os
        import shutil

        shutil.rmtree("/tmp/bass_ntff", ignore_errors=True)
        os.makedirs("/tmp/bass_ntff", exist_ok=True)
        kwargs["tmpdir"] = "/tmp/bass_ntff"
    res = bass_utils.run_bass_kernel_spmd(
        nc, in_maps, core_ids=list(range(NCORES)), trace=_trace, **kwargs
    )
    # outT (C, ROWS) fp16 -> (BL, P, C) fp32 per core
    outs = []
    for c in range(NCORES):
        oT = np.asarray(res.results[c]["outT"], np.float32)   # (768, 1568)
        outs.append(oT.reshape(C, BL, P).transpose(1, 2, 0))
    out = np.concatenate(outs, axis=0).reshape(B, P, C)
    if _trace:
        kernel.last_results = res
    return out
